# revision 26
# baseline (speedup 1.0000x reference)
"""BU-Net loss (weighted CE + dice) Trainium2 kernel.

Math
----
reference(pred[N,C,H,W] f32, target[N,H,W] i64) with C=4 classes:
  counts[k] = global histogram of target; cw = 1/(counts+eps); w(px) = cw[t(px)]
  wce  = -mean_n( sum_px(w*(pred_t - lse)) / sum_px(w) ),  lse = logsumexp_c pred
  dice = mean_{n,c}(1 - (2*I+1)/(U+1)),
         I[n,c] = sum_px pred_c*t*w,  U[n,c] = sum_px pred_c*w + sum_px t*w

Everything is linear in per-class masked sums, so the device only computes
  P[n,c,k]   = sum_px pred_c * 1[t==k]     (16 values / image)
  Lambda[n,k]= sum_px lse * 1[t==k]        (4 values / image)
  count[n,k]                                (host histogram of the target)
and the host combines in float64 (w and t*w are constant per class):
  sum w = sum_k cw_k count_k;  sum w*pred_t = sum_k cw_k P[k,k]
  sum w*lse = sum_k cw_k Lambda_k
  I[c] = sum_k k*cw_k*P[c,k],  U[c] = sum_k cw_k*P[c,k] + sum_k k*cw_k*count_k
No on-device collective is needed: the "all-reduce" of class counts happens
on host (target is 32x smaller than pred), and per-core partials are tiny.

Device program per core (2 images; batch is data-parallel over 8 cores):
  - inputs: pred as bf16, block-interleaved [P, NBLK, C, BLK] so each
    128-column block has all 4 channels contiguous; target as bf16 plane.
    (bf16 pred perturbs the loss ~1e-5: errors average over 262k px/image.)
  - masks m_k = is_equal(t, k) on DVE (bf16, 4x perf mode)
  - P[c,k] via TensorE: per 128-col block b, PSUM_k += m_k[:,b]^T @ pred[:,b]
    accumulated over the 16 blocks; the wanted sums are the traces of the
    128x128 sub-blocks, extracted on host from a bf16 PSUM dump (PSUM is
    copied to SBUF by ScalarE; diagonals are host-side numpy).
  - lse: ScalarE Exp over the whole interleaved plane (1 op), DVE bf16 adds,
    ScalarE Ln, with accum_out giving sum(lse) per partition for free.
  - Lambda_k (k<3) via fused DVE scalar_tensor_tensor:
      out=(t is_equal k) mult lse, accum_out = per-partition sum;
    Lambda_3 = sum(lse) - Lambda_0..2 on host.
  - All big input DMAs are chunked across HWDGE queues (one dma_start runs
    on one queue at ~31 GB/s); output DMAs go through SWDGE (Pool engine)
    to keep the SP sequencer off the critical path.
The exp/add/ln/STT chain is pipelined by half-plane so it overlaps the
input DMAs and PE work instead of forming a serial tail.
Measured: ~34 us device time per pass steady-state (paired repeat-delta;
Tile cost model predicts 44 us single-shot makespan, PE/DVE/ACT all ~27-29 us
busy); loss rel err vs the f32 reference ~3.5e-5.
"""

import sys

for _p in ("/opt/trn_rl_repo",):
    if _p not in sys.path:
        sys.path.insert(0, _p)

from contextlib import ExitStack

import ml_dtypes
import numpy as np

import concourse.bass as bass
import concourse.mybir as mybir
import concourse.tile as tile
from concourse import bacc, bass2jax

N, C, H, W = 16, 4, 512, 512
EPS = 1e-6
SMOOTH = 1.0
NCORES = 8
IMG = N // NCORES  # images per core
P = 128            # partitions
FREE = (H * W) // P  # 2048 free columns per plane
NBLK = 16          # 128-column blocks per plane
BLK = 128

_BF16 = mybir.dt.bfloat16
_FP16 = mybir.dt.float16
_FP32 = mybir.dt.float32

LAST_RESULTS = None  # BassKernelResults of the most recent run (for test.py)


def _f32_to_bf16(x: np.ndarray) -> np.ndarray:
    """Round-to-nearest-even f32 -> bf16 without needing jax."""
    u = np.ascontiguousarray(x, dtype=np.float32).view(np.uint32)
    r = (u + np.uint32(0x7FFF) + ((u >> np.uint32(16)) & np.uint32(1))) >> np.uint32(16)
    return r.astype(np.uint16).view(ml_dtypes.bfloat16)


def _make_pools(ctx: ExitStack, tc: "tile.TileContext"):
    return dict(
        inpool=ctx.enter_context(tc.tile_pool(name="in", bufs=3)),
        mpool=ctx.enter_context(tc.tile_pool(name="masks", bufs=2)),
        work=ctx.enter_context(tc.tile_pool(name="work", bufs=2)),
        psump=ctx.enter_context(tc.tile_pool(name="psum", bufs=8, space="PSUM")),
        accp=ctx.enter_context(tc.tile_pool(name="acc", bufs=2)),
        # dedicated pool, one slot per (image, k): no slot-reuse waits on the
        # PSUM->SBUF copies (walrus rejects compute instructions with >2 sem waits)
        psbp=ctx.enter_context(tc.tile_pool(name="psb", bufs=2 * C)),
    )


def _body(ctx: ExitStack, tc: "tile.TileContext", pred_d, t_d, pdump_d, lam_d,
          pools=None):
    nc = tc.nc
    fa = mybir.ActivationFunctionType
    alu = mybir.AluOpType

    p = pools or _make_pools(ctx, tc)
    inpool, mpool, work, psump, accp, psbp = (
        p["inpool"], p["mpool"], p["work"], p["psump"], p["accp"], p["psbp"])

    preds, tts = [], []
    # phase A: loads, masks, matmuls, psum dumps (per image)
    for i in range(IMG):
        pred = inpool.tile([P, NBLK, C, BLK], _BF16, tag="pred")
        tt = inpool.tile([P, NBLK, BLK], _BF16, tag="t")
        preds.append(pred)
        tts.append(tt)
        # fine-grained input chunks: all 8 HWDGE queues fill in parallel and
        # the first blocks land early so PE can start ~5us in, not ~15us
        # (one dma_start = one queue; SP pays ~0.4us dispatch per DMA)
        for sj in range(0, NBLK, 4):
            nc.sync.dma_start(tt[:, sj:sj + 4], t_d[i, :, sj:sj + 4])
        for sj in range(0, NBLK, 2):
            nc.sync.dma_start(pred[:, sj:sj + 2], pred_d[i, :, sj:sj + 2])

        # masks per half-plane so the first 8 blocks of matmuls only wait on
        # the first half of the target plane
        masks = []
        for k in range(C):
            mk = mpool.tile([P, NBLK, BLK], _BF16, tag=f"m{k}")
            half = NBLK // 2
            nc.vector.tensor_scalar(mk[:, :half], tt[:, :half], float(k), None, alu.is_equal)
            nc.vector.tensor_scalar(mk[:, half:], tt[:, half:], float(k), None, alu.is_equal)
            masks.append(mk)

        # P[c,k]: PSUM_k[j', c*128+j''] += sum_p m_k[p,b*128+j'] * pred_c[p,b*128+j'']
        for k in range(C):
            ps = psump.tile([P, C * BLK], _FP32, tag="ps")
            for b in range(NBLK):
                nc.tensor.matmul(
                    ps[:],
                    lhsT=masks[k][:, b, :],
                    rhs=pred[:, b],
                    start=(b == 0),
                    stop=(b == NBLK - 1),
                )
            sb = psbp.tile([P, C * BLK], _BF16, tag="psb")
            if k % 2 == 0:
                nc.scalar.copy(sb[:], ps[:])
            else:
                nc.vector.tensor_copy(sb[:], ps[:])
            nc.gpsimd.dma_start(pdump_d[i, k], sb[:])

    # per-image lse + Lambda chain, pipelined by half-plane: each half's
    # exp/add/ln/STT starts as soon as that half of pred has arrived, so the
    # chain overlaps the DMAs and PE work instead of forming a serial tail
    HALF = NBLK // 2
    for i in range(IMG):
        e = work.tile([P, NBLK, C, BLK], _BF16, tag="e")
        s01 = work.tile([P, NBLK, BLK], _BF16, tag="s01")
        s23 = work.tile([P, NBLK, BLK], _BF16, tag="s23")
        s = work.tile([P, NBLK, BLK], _BF16, tag="s")
        lse = work.tile([P, NBLK, BLK], _BF16, tag="lse")
        sumlse = [None, None]
        accs = {}
        for h in range(2):
            sl = slice(h * HALF, (h + 1) * HALF)
            nc.scalar.activation(e[:, sl], preds[i][:, sl], fa.Exp)
            nc.vector.tensor_add(s01[:, sl], e[:, sl, 0, :], e[:, sl, 1, :])
            nc.vector.tensor_add(s23[:, sl], e[:, sl, 2, :], e[:, sl, 3, :])
            nc.vector.tensor_add(s[:, sl], s01[:, sl], s23[:, sl])
            sl_acc = accp.tile([P, 1], _FP32, tag=f"sumlse{h}")
            sumlse[h] = sl_acc
            # accum_out gives sum(lse-half) per partition for free
            nc.scalar.activation(lse[:, sl], s[:, sl], fa.Ln, accum_out=sumlse[h][:])
            for k in range(C - 1):
                so = work.tile([P, NBLK // 2, BLK], _BF16, tag="sttout")
                acc = accp.tile([P, 1], _FP32, tag=f"acc{k}{h}")
                nc.vector.scalar_tensor_tensor(
                    out=so[:], in0=tts[i][:, sl], scalar=float(k), in1=lse[:, sl],
                    op0=alu.is_equal, op1=alu.mult,
                    accum_out=acc[:],
                )
                accs[(k, h)] = acc
        # combine halves (tiny [128,1] adds) and ship; host recovers
        # Lambda_3 = sum(lse) - Lambda_0 - Lambda_1 - Lambda_2
        stot = accp.tile([P, 1], _FP32, tag="stot")
        nc.vector.tensor_add(stot[:], sumlse[0][:], sumlse[1][:])
        nc.gpsimd.dma_start(lam_d[i, C - 1], stot[:])
        for k in range(C - 1):
            ktot = accp.tile([P, 1], _FP32, tag=f"ktot{k}")
            nc.vector.tensor_add(ktot[:], accs[(k, 0)][:], accs[(k, 1)][:])
            nc.gpsimd.dma_start(lam_d[i, k], ktot[:])


_CACHED = None


def _get_nc():
    global _CACHED
    if _CACHED is None:
        nc = bacc.Bacc("TRN2", target_bir_lowering=False, debug=False)
        pred_d = nc.dram_tensor(
            "pred_il", [IMG, P, NBLK, C, BLK], _BF16, kind="ExternalInput"
        ).ap()
        t_d = nc.dram_tensor(
            "t_bf", [IMG, P, NBLK, BLK], _BF16, kind="ExternalInput"
        ).ap()
        pdump_d = nc.dram_tensor(
            "pdump", [IMG, C, P, C * BLK], _BF16, kind="ExternalOutput"
        ).ap()
        lam_d = nc.dram_tensor(
            "lam", [IMG, C, P, 1], _FP32, kind="ExternalOutput"
        ).ap()
        with tile.TileContext(nc) as tc, ExitStack() as ctx:
            _body(ctx, tc, pred_d, t_d, pdump_d, lam_d)
        nc.compile()
        _CACHED = nc
    return _CACHED


def _prep_inputs(pred: np.ndarray, target: np.ndarray):
    """Host-side shard prep + histogram ("all-reduce" of class counts)."""
    pred = np.ascontiguousarray(pred, dtype=np.float32)
    tgt = np.clip(target, 0, C - 1)

    counts_nk = np.stack(
        [np.bincount(tgt[n].ravel().astype(np.int64), minlength=C) for n in range(N)]
    ).astype(np.float64)
    cw = 1.0 / (counts_nk.sum(0) + EPS)  # [C] float64

    # pixel (p, b, j): hw_flat = p*2048 + b*128 + j
    pred_bf = _f32_to_bf16(pred)  # cast first (halves the transpose traffic)
    predr = pred_bf.reshape(N, C, P, NBLK, BLK).transpose(0, 2, 3, 1, 4)
    pred_il = np.ascontiguousarray(predr)  # [N,P,NBLK,C,BLK]
    tr = tgt.reshape(N, P, NBLK, BLK)
    t_bf = tr.astype(ml_dtypes.bfloat16)

    in_maps = [
        {
            "pred_il": pred_il[IMG * c : IMG * (c + 1)],
            "t_bf": t_bf[IMG * c : IMG * (c + 1)],
        }
        for c in range(NCORES)
    ]
    return in_maps, counts_nk, cw


def _combine(results, counts_nk, cw) -> np.float32:
    """float64 host reduction of the per-core partial sums."""
    Pmat = np.zeros((N, C, C))  # [n, c, k]
    WL = np.zeros((N,))
    ks = np.arange(C, dtype=np.float64)
    for core in range(NCORES):
        pd = np.asarray(results[core]["pdump"], dtype=np.float64)  # [IMG,C,P,C*BLK]
        lam = np.asarray(results[core]["lam"], dtype=np.float64)  # [IMG,C,P,1]
        for ii in range(IMG):
            n = core * IMG + ii
            for k in range(C):
                for c in range(C):
                    Pmat[n, c, k] = np.trace(pd[ii, k, :, c * BLK : (c + 1) * BLK])
            lsum = lam[ii, :, :, 0].sum(axis=1)  # [C]; last entry = sum(lse)
            lsum[C - 1] = lsum[C - 1] - lsum[: C - 1].sum()
            WL[n] = lsum @ cw

    den = counts_nk @ cw                      # [n] = sum w
    twsum = counts_nk @ (ks * cw)             # [n] = sum t*w
    A = np.einsum("nkk,k->n", Pmat, cw)       # [n] = sum w*pred_t
    wce = -np.mean((A - WL) / den)
    I = np.einsum("nck,k->nc", Pmat, ks * cw)
    U = np.einsum("nck,k->nc", Pmat, cw) + twsum[:, None]
    dice = np.mean(1.0 - (2.0 * I + SMOOTH) / (U + SMOOTH))
    return np.float32(wce + dice)


_RUNNER = None


def _get_runner():
    """Cached jit(shard_map) runner over 8 cores (mirrors
    bass2jax.run_bass_via_pjrt's multi-core path, but built once)."""
    global _RUNNER
    if _RUNNER is not None:
        return _RUNNER
    import jax
    from jax.experimental.shard_map import shard_map
    from jax.sharding import Mesh, PartitionSpec

    nc = _get_nc()
    bass2jax.install_neuronx_cc_hook()

    in_names, out_names, out_avals, zero_outs = [], [], [], []
    partition_name = nc.partition_id_tensor.name if nc.partition_id_tensor else None
    for alloc in nc.m.functions[0].allocations:
        if not isinstance(alloc, mybir.MemoryLocationSet):
            continue
        name = alloc.memorylocations[0].name
        if alloc.kind == "ExternalInput":
            if name != partition_name:
                in_names.append(name)
        elif alloc.kind == "ExternalOutput":
            shape = tuple(alloc.tensor_shape)
            dtype = mybir.dt.np(alloc.dtype)
            out_avals.append(jax.core.ShapedArray(shape, dtype))
            out_names.append(name)
            zero_outs.append(np.zeros(shape, dtype))
    n_params = len(in_names)
    n_outs = len(out_avals)
    all_in_names = list(in_names) + list(out_names)
    if partition_name is not None:
        all_in_names.append(partition_name)

    def _bdy(*args):
        operands = list(args)
        if partition_name is not None:
            operands.append(bass2jax.partition_id_tensor())
        return tuple(
            bass2jax._bass_exec_p.bind(
                *operands,
                out_avals=tuple(out_avals),
                in_names=tuple(all_in_names),
                out_names=tuple(out_names),
                lowering_input_output_aliases=(),
                sim_require_finite=True,
                sim_require_nnan=True,
                nc=nc,
            )
        )

    devices = jax.devices()[:NCORES]
    mesh = Mesh(np.asarray(devices), ("core",))
    donate = tuple(range(n_params, n_params + n_outs))
    sharded = jax.jit(
        shard_map(
            _bdy,
            mesh=mesh,
            in_specs=(PartitionSpec("core"),) * (n_params + n_outs),
            out_specs=(PartitionSpec("core"),) * n_outs,
            check_rep=False,
        ),
        donate_argnums=donate,
        keep_unused=True,
    )
    _RUNNER = (sharded, in_names, out_names, out_avals, zero_outs)
    return _RUNNER


def _run_device(in_maps):
    sharded, in_names, out_names, out_avals, zero_outs = _get_runner()
    concat_in = [
        np.concatenate([np.asarray(in_maps[c][name]) for c in range(NCORES)], axis=0)
        for name in in_names
    ]
    concat_zeros = [
        np.zeros((NCORES * z.shape[0], *z.shape[1:]), z.dtype) for z in zero_outs
    ]
    out_arrs = sharded(*concat_in, *concat_zeros)
    return [
        {
            name: np.asarray(out_arrs[i]).reshape(NCORES, *out_avals[i].shape)[c]
            for i, name in enumerate(out_names)
        }
        for c in range(NCORES)
    ]


def kernel(pred: np.ndarray, target: np.ndarray) -> np.ndarray:
    in_maps, counts_nk, cw = _prep_inputs(np.asarray(pred), np.asarray(target))
    results = _run_device(in_maps)
    return _combine(results, counts_nk, cw)
